# revision 42
# baseline (speedup 1.0000x reference)
"""Causal self-attention (B=2, T=2048, E=1024, H=16, d_k=64) on 8 TRN2 cores.

Tensor-parallel over heads: core c owns heads 2c, 2c+1 (feature slice
c*128:(c+1)*128 of the QKV projections and of the Wo contraction dim).
Each core computes a partial output [1024, 4096] (embd-major, bf16); the
host sums the 8 partials in fp32, adds bo, and transposes back.

All matmul operands are bf16 (1 col/cycle on the PE, half the LDWEIGHTS
cost of fp32r); accumulation stays fp32 in PSUM. Attention blocks above
the causal diagonal are skipped at 128-column granularity, the diagonal
128x128 sub-block is masked with a multiplicative tril mask on GpSimd
after the exp. Softmax denominators are accumulated via a 64-wide ones
block in the PV stationary operand, so the reciprocal lands already
broadcast across partitions 64:128 of the PSUM accumulator.
"""

import numpy as np

B = 2
T = 2048
E = 1024
F = 128          # per-core QKV features (2 heads x 64)
DK = 64
N_CORES = 8
TT = B * T       # flattened tokens
IC = 512         # query chunk (PSUM bank width in fp32)
JC = 128         # key block
NTB = TT // JC   # 32 token blocks of 128

_CACHE = {}


def _build_program(debug_taps=False):
    import concourse.mybir as mybir
    import concourse.tile as tile
    from concourse import bacc

    f32 = mybir.dt.float32
    bf16 = mybir.dt.bfloat16
    Act = mybir.ActivationFunctionType

    n_ec = E // 128          # 8 contraction chunks for the projections
    n_icb = T // IC          # 4 query chunks per batch
    n_jcb = T // JC          # 16 key blocks per batch

    nc = bacc.Bacc("TRN2", target_bir_lowering=False, debug=False)

    # x arrives already transposed on the host: [embd, t_total]
    x_ap = nc.dram_tensor("x", [E, TT], bf16, kind="ExternalInput").ap()
    wqT = nc.dram_tensor("wqT", [E, F], bf16, kind="ExternalInput").ap()
    wkT = nc.dram_tensor("wkT", [E, F], bf16, kind="ExternalInput").ap()
    wvT = nc.dram_tensor("wvT", [E, F], bf16, kind="ExternalInput").ap()
    woT = nc.dram_tensor("woT", [F, E], bf16, kind="ExternalInput").ap()
    bq_ap = nc.dram_tensor("bq", [F], f32, kind="ExternalInput").ap()
    bk_ap = nc.dram_tensor("bk", [F], f32, kind="ExternalInput").ap()
    bv_ap = nc.dram_tensor("bv", [F], f32, kind="ExternalInput").ap()
    tril_ap = nc.dram_tensor("tril", [JC, JC], bf16, kind="ExternalInput").ap()
    out_ap = nc.dram_tensor("partial", [E, TT], bf16, kind="ExternalOutput").ap()
    if debug_taps:
        dbg_qt = nc.dram_tensor("dbg_qt", [128, TT], bf16, kind="ExternalOutput").ap()
        dbg_kt = nc.dram_tensor("dbg_kt", [128, TT], bf16, kind="ExternalOutput").ap()
        dbg_v1 = nc.dram_tensor("dbg_v1", [128, NTB, 256], bf16, kind="ExternalOutput").ap()
        dbg_yt = nc.dram_tensor("dbg_yt", [128, TT], bf16, kind="ExternalOutput").ap()

    with tile.TileContext(nc) as tc:
        with (
            tc.tile_pool(name="const", bufs=1) as constp,
            tc.tile_pool(name="persist", bufs=1) as persist,
            tc.tile_pool(name="xb", bufs=2) as xbp,
            tc.tile_pool(name="vt", bufs=2) as vtp,
            tc.tile_pool(name="pt", bufs=8) as ptp,
            tc.tile_pool(name="work", bufs=4) as work,
            tc.tile_pool(name="outs", bufs=8) as outsp,
            tc.tile_pool(name="ps_a", bufs=4, space="PSUM") as ps_a,
            tc.tile_pool(name="ps_y", bufs=4, space="PSUM") as ps_y,
        ):
            # ---- constants; k-projection weights first, then batch-0 x,
            # so the first projection round unblocks as early as possible ----
            wk_sb = constp.tile([128, n_ec, F], bf16, tag="wk")
            nc.sync.dma_start(wk_sb[:], wkT.rearrange("(a p) f -> p a f", p=128))
            bk_sb = constp.tile([128, 1], f32, tag="bk")
            nc.sync.dma_start(bk_sb[:], bk_ap[:])

            xt_re = x_ap.rearrange("(a p) t -> p a t", p=128)
            xbs = []
            for b in range(B):
                xb = xbp.tile([128, n_ec, T], bf16, tag="xb", name=f"xb{b}")
                xbs.append(xb)
            # batch-0 first chunk split along ec so the first matmul
            # unblocks early
            for piece in range(4):
                nc.sync.dma_start(
                    xbs[0][:, piece * 2 : (piece + 1) * 2, 0:IC],
                    xt_re[:, piece * 2 : (piece + 1) * 2, 0:IC],
                )
            for tch in range(1, n_icb):
                for half in range(2):
                    nc.sync.dma_start(
                        xbs[0][:, half * 4 : (half + 1) * 4, tch * IC : (tch + 1) * IC],
                        xt_re[:, half * 4 : (half + 1) * 4, tch * IC : (tch + 1) * IC],
                    )

            wq_sb = constp.tile([128, n_ec, F], bf16, tag="wq")
            nc.sync.dma_start(wq_sb[:], wqT.rearrange("(a p) f -> p a f", p=128))
            wv_sb = constp.tile([128, n_ec, F], bf16, tag="wv")
            nc.sync.dma_start(wv_sb[:], wvT.rearrange("(a p) f -> p a f", p=128))
            wo_sb = constp.tile([128, E], bf16, tag="wo")
            nc.sync.dma_start(wo_sb[:], woT)
            bq_sb = constp.tile([128, 1], f32, tag="bq")
            nc.sync.dma_start(bq_sb[:], bq_ap[:])
            bv_sb = constp.tile([128, 1], f32, tag="bv")
            nc.sync.dma_start(bv_sb[:], bv_ap[:])
            tril_sb = constp.tile([128, JC], bf16, tag="tril")
            nc.sync.dma_start(tril_sb[:], tril_ap)
            ones_f32 = constp.tile([128, 1], f32, tag="ones_f32")
            nc.vector.memset(ones_f32[:], 1.0)
            ones_row = ones_f32[:, 0:1].broadcast_to([128, IC])

            for tch in range(n_icb):
                nc.sync.dma_start(
                    xbs[1][:, :, tch * IC : (tch + 1) * IC],
                    xt_re[:, :, T + tch * IC : T + (tch + 1) * IC],
                )

            # ---- persistent activations (all bf16) ----
            qt_sb = persist.tile([128, TT], bf16, tag="qt")    # [f, t]
            kt_sb = persist.tile([128, TT], bf16, tag="kt")    # [f, t]
            # V per token-block: head h occupies cols h*128 .. h*128+64 (V)
            # and h*128+64 .. (h+1)*128 (ones -> replicated denominator)
            v1_sb = persist.tile([128, NTB, 256], bf16, tag="v1")
            yt_sb = persist.tile([128, TT], bf16, tag="yt")    # [e', t] normalized

            for h in range(2):
                nc.gpsimd.memset(v1_sb[:, :, h * 128 + 64 : (h + 1) * 128], 1.0)

            def emit_d(t0):
                # output projection for the 512-token chunk at t0
                for eo in range(E // 128):
                    pso = ps_a.tile([128, IC], f32, tag="psb", name="pso")
                    nc.tensor.matmul(
                        pso[:],
                        wo_sb[:, eo * 128 : (eo + 1) * 128],
                        yt_sb[:, t0 : t0 + IC],
                        start=True,
                        stop=True,
                    )
                    ob = outsp.tile([128, IC], bf16, tag="ob")
                    if eo % 2 == 0:
                        nc.vector.tensor_copy(ob[:], pso[:])
                    else:
                        nc.scalar.activation(
                            ob[:], pso[:], mybir.ActivationFunctionType.Copy
                        )
                    nc.sync.dma_start(
                        out_ap[eo * 128 : (eo + 1) * 128, t0 : t0 + IC], ob[:]
                    )

            d_queue = []
            for b in range(B):
                tb = b * T

                # ---- phase B: project this batch's tokens to QT/KT/VT ----
                xb = xbs[b]
                vt = vtp.tile([128, T], bf16, tag="vt")
                for w_sb, b_sb, kind in (
                    (wk_sb, bk_sb, "k"),
                    (wq_sb, bq_sb, "q"),
                    (wv_sb, bv_sb, "v"),
                ):
                    pss = [
                        ps_a.tile([128, IC], f32, tag="psb", name="psb")
                        for i in range(n_icb)
                    ]
                    for tch in range(n_icb):
                        for ec in range(n_ec):
                            nc.tensor.matmul(
                                pss[tch][:],
                                w_sb[:, ec, :],
                                xb[:, ec, tch * IC : (tch + 1) * IC],
                                start=(ec == 0),
                                stop=(ec == n_ec - 1),
                            )
                        if kind == "q":
                            dst = qt_sb[:, tb + tch * IC : tb + (tch + 1) * IC]
                        elif kind == "k":
                            dst = kt_sb[:, tb + tch * IC : tb + (tch + 1) * IC]
                        else:
                            dst = vt[:, tch * IC : (tch + 1) * IC]
                        nc.vector.scalar_tensor_tensor(
                            dst, pss[tch][:], b_sb[:], ones_row,
                            op0=mybir.AluOpType.add, op1=mybir.AluOpType.mult,
                        )
                        if kind == "v":
                            # V back to natural [t, f] layout with the ones
                            # blocks: one whole-chunk DMA crossbar transpose
                            # (extra out dims fold into the partition dim),
                            # then one strided copy into the per-head slots
                            tci0 = b * (T // 128) + tch * (IC // 128)
                            vtT = work.tile([128, IC // 128, 128], bf16, tag="vtT")
                            nc.sync.dma_start_transpose(
                                vtT[:], vt[:, tch * IC : (tch + 1) * IC]
                            )
                            dst = v1_sb[
                                :, tci0 : tci0 + IC // 128, :
                            ].rearrange("p t (h c) -> p t h c", h=2)
                            nc.vector.tensor_copy(
                                dst[:, :, :, 0:64],
                                vtT[:].rearrange("p t (h c) -> p t h c", h=2),
                            )
                    if kind == "k" and d_queue:
                        # previous batch's last out-projection chunk, deferred
                        # so its PSUM drains overlap this projection round
                        emit_d(d_queue.pop(0))

                # ---- phase C/D interleaved: attention, then out-proj per
                # pair of finished query chunks ----
                for h in range(2):
                    r0 = h * DK
                    yps = [
                        ps_y.tile([128, IC], f32, tag="yp", name=f"yp{i}")
                        for i in range(n_icb)
                    ]

                    def flush_pvs(pend):
                        jc, tci, pts = pend
                        for ic, pt, w, cs in pts:
                            nc.tensor.matmul(
                                yps[ic][:, cs:IC],
                                v1_sb[:, tci, h * 128 : (h + 1) * 128],
                                pt[:, :w],
                                start=(jc == 0),
                                stop=(jc == 4 * ic + 3),
                                skip_group_check=True,
                            )
                        if jc % 4 == 3:
                            # yps[jc//4] is complete: normalize into yt
                            icd = jc // 4
                            den = work.tile([DK, IC], f32, tag="den")
                            nc.vector.tensor_copy(den[:], yps[icd][DK:128, :])
                            rcp = work.tile([DK, IC], f32, tag="rcp")
                            nc.vector.reciprocal_approx_fast(rcp[:], den[:])
                            nc.vector.tensor_mul(
                                yt_sb[r0 : r0 + DK, tb + icd * IC : tb + (icd + 1) * IC],
                                yps[icd][0:DK, :],
                                rcp[:],
                            )
                            if h == 1:
                                d_queue.append(tb + icd * IC)

                    pending = None
                    for jc in range(n_jcb):
                        ic0 = jc // 4
                        o = jc % 4
                        tci = b * (T // 128) + jc
                        pts = []
                        for ic in range(ic0, n_icb):
                            diag = ic == ic0
                            cs = o * JC if diag else 0
                            w = IC - cs
                            q0 = tb + ic * IC + cs
                            st = ps_a.tile([128, IC], f32, tag="psb", name="st")
                            nc.tensor.matmul(
                                st[:, :w],
                                kt_sb[r0 : r0 + DK, tb + jc * JC : tb + (jc + 1) * JC],
                                qt_sb[r0 : r0 + DK, q0 : q0 + w],
                                start=True,
                                stop=True,
                            )
                            pt = ptp.tile([128, IC], bf16, tag="pt")
                            nc.scalar.activation(
                                pt[:, :w], st[:, :w], Act.Exp, scale=0.125
                            )
                            if diag:
                                # mask the diagonal 128x128 sub-block
                                nc.gpsimd.tensor_mul(
                                    pt[:, 0:JC], pt[:, 0:JC], tril_sb[:]
                                )
                            pts.append((ic, pt, w, cs))
                        if d_queue:
                            # drain a deferred out-projection chunk behind a
                            # fresh buffer of QK work
                            emit_d(d_queue.pop(0))
                        if pending is not None:
                            flush_pvs(pending)
                        pending = (jc, tci, pts)
                    flush_pvs(pending)

            while d_queue:
                emit_d(d_queue.pop(0))

            if debug_taps:
                nc.sync.dma_start(dbg_qt[:], qt_sb[:])
                nc.sync.dma_start(dbg_kt[:], kt_sb[:])
                nc.sync.dma_start(dbg_v1[:], v1_sb[:])
                nc.sync.dma_start(dbg_yt[:], yt_sb[:])

    nc.compile()
    return nc


def _get_program():
    if "nc" not in _CACHE:
        _CACHE["nc"] = _build_program()
    return _CACHE["nc"]


def _prepare_in_maps(inputs):
    import ml_dtypes

    bf16 = ml_dtypes.bfloat16
    x = np.ascontiguousarray(
        np.asarray(inputs["x"], dtype=np.float32).reshape(TT, E).T
    ).astype(bf16)
    Wq = np.asarray(inputs["Wq"], dtype=np.float32)
    Wk = np.asarray(inputs["Wk"], dtype=np.float32)
    Wv = np.asarray(inputs["Wv"], dtype=np.float32)
    Wo = np.asarray(inputs["Wo"], dtype=np.float32)
    bq = np.asarray(inputs["bq"], dtype=np.float32)
    bk = np.asarray(inputs["bk"], dtype=np.float32)
    bv = np.asarray(inputs["bv"], dtype=np.float32)

    j = np.arange(JC)[:, None]
    i = np.arange(JC)[None, :]
    tril = (j <= i).astype(bf16)  # key j visible to query i

    in_maps = []
    for c in range(N_CORES):
        sl = slice(c * F, (c + 1) * F)
        in_maps.append(
            {
                "x": x,
                "wqT": np.ascontiguousarray(Wq[sl].T).astype(bf16),
                "wkT": np.ascontiguousarray(Wk[sl].T).astype(bf16),
                "wvT": np.ascontiguousarray(Wv[sl].T).astype(bf16),
                "woT": np.ascontiguousarray(Wo[:, sl].T).astype(bf16),
                "bq": np.ascontiguousarray(bq[sl]),
                "bk": np.ascontiguousarray(bk[sl]),
                "bv": np.ascontiguousarray(bv[sl]),
                "tril": tril,
            }
        )
    return in_maps


def kernel(x, Wq, bq, Wk, bk, Wv, bv, Wo, bo):
    from concourse.bass_utils import run_bass_kernel_spmd

    nc = _get_program()
    bo = np.asarray(bo, dtype=np.float32)
    in_maps = _prepare_in_maps(
        {"x": x, "Wq": Wq, "bq": bq, "Wk": Wk, "bk": bk,
         "Wv": Wv, "bv": bv, "Wo": Wo, "bo": bo}
    )

    res = run_bass_kernel_spmd(nc, in_maps, core_ids=list(range(N_CORES)))
    out = np.zeros((E, TT), dtype=np.float64)
    for c in range(N_CORES):
        out += res.results[c]["partial"].astype(np.float32)
    out = out.T + bo[None, :]
    return out.reshape(B, T, E).astype(np.float32)


# revision 43
# speedup vs baseline: 1.7810x; 1.7810x over previous
"""Causal self-attention (B=2, T=2048, E=1024, H=16, d_k=64) on 8 TRN2 cores.

Tensor-parallel over heads: core c owns heads 2c, 2c+1 (feature slice
c*128:(c+1)*128 of the QKV projections and of the Wo contraction dim).
Each core computes a partial output [1024, 4096] (embd-major, bf16); the
host sums the 8 partials in fp32, adds bo, and transposes back.

All matmul operands are bf16 (1 col/cycle on the PE, half the LDWEIGHTS
cost of fp32r); accumulation stays fp32 in PSUM. Attention blocks above
the causal diagonal are skipped at 128-column granularity, the diagonal
128x128 sub-block is masked with a multiplicative tril mask on GpSimd
after the exp. Softmax denominators are accumulated via a 64-wide ones
block in the PV stationary operand, so the reciprocal lands already
broadcast across partitions 64:128 of the PSUM accumulator.
"""

import numpy as np

B = 2
T = 2048
E = 1024
F = 128          # per-core QKV features (2 heads x 64)
DK = 64
N_CORES = 8
TT = B * T       # flattened tokens
IC = 512         # query chunk (PSUM bank width in fp32)
JC = 128         # key block
NTB = TT // JC   # 32 token blocks of 128

_CACHE = {}


def _build_program(debug_taps=False):
    import concourse.mybir as mybir
    import concourse.tile as tile
    from concourse import bacc

    f32 = mybir.dt.float32
    bf16 = mybir.dt.bfloat16
    Act = mybir.ActivationFunctionType

    n_ec = E // 128          # 8 contraction chunks for the projections
    n_icb = T // IC          # 4 query chunks per batch
    n_jcb = T // JC          # 16 key blocks per batch

    nc = bacc.Bacc("TRN2", target_bir_lowering=False, debug=False)

    # x arrives already transposed on the host: [embd, t_total]
    x_ap = nc.dram_tensor("x", [E, TT], bf16, kind="ExternalInput").ap()
    wqT = nc.dram_tensor("wqT", [E, F], bf16, kind="ExternalInput").ap()
    wkT = nc.dram_tensor("wkT", [E, F], bf16, kind="ExternalInput").ap()
    wvT = nc.dram_tensor("wvT", [E, F], bf16, kind="ExternalInput").ap()
    woT = nc.dram_tensor("woT", [F, E], bf16, kind="ExternalInput").ap()
    bq_ap = nc.dram_tensor("bq", [F], f32, kind="ExternalInput").ap()
    bk_ap = nc.dram_tensor("bk", [F], f32, kind="ExternalInput").ap()
    bv_ap = nc.dram_tensor("bv", [F], f32, kind="ExternalInput").ap()
    tril_ap = nc.dram_tensor("tril", [JC, JC], bf16, kind="ExternalInput").ap()
    out_ap = nc.dram_tensor("partial", [E, TT], bf16, kind="ExternalOutput").ap()
    if debug_taps:
        dbg_qt = nc.dram_tensor("dbg_qt", [128, TT], bf16, kind="ExternalOutput").ap()
        dbg_kt = nc.dram_tensor("dbg_kt", [128, TT], bf16, kind="ExternalOutput").ap()
        dbg_v1 = nc.dram_tensor("dbg_v1", [128, NTB, 256], bf16, kind="ExternalOutput").ap()
        dbg_yt = nc.dram_tensor("dbg_yt", [128, TT], bf16, kind="ExternalOutput").ap()

    with tile.TileContext(nc) as tc:
        with (
            tc.tile_pool(name="const", bufs=1) as constp,
            tc.tile_pool(name="persist", bufs=1) as persist,
            tc.tile_pool(name="xb", bufs=2) as xbp,
            tc.tile_pool(name="vt", bufs=2) as vtp,
            tc.tile_pool(name="pt", bufs=12) as ptp,
            tc.tile_pool(name="work", bufs=4) as work,
            tc.tile_pool(name="outs", bufs=8) as outsp,
            tc.tile_pool(name="ps_a", bufs=4, space="PSUM") as ps_a,
            tc.tile_pool(name="ps_y", bufs=4, space="PSUM") as ps_y,
        ):
            # ---- constants; k-projection weights first, then batch-0 x,
            # so the first projection round unblocks as early as possible ----
            wk_sb = constp.tile([128, n_ec, F], bf16, tag="wk")
            nc.sync.dma_start(wk_sb[:], wkT.rearrange("(a p) f -> p a f", p=128))
            bk_sb = constp.tile([128, 1], f32, tag="bk")
            nc.sync.dma_start(bk_sb[:], bk_ap[:])

            xt_re = x_ap.rearrange("(a p) t -> p a t", p=128)
            xbs = []
            for b in range(B):
                xb = xbp.tile([128, n_ec, T], bf16, tag="xb", name=f"xb{b}")
                xbs.append(xb)
            # batch-0 first chunk split along ec so the first matmul
            # unblocks early
            for piece in range(4):
                nc.sync.dma_start(
                    xbs[0][:, piece * 2 : (piece + 1) * 2, 0:IC],
                    xt_re[:, piece * 2 : (piece + 1) * 2, 0:IC],
                )
            for tch in range(1, n_icb):
                for half in range(2):
                    nc.sync.dma_start(
                        xbs[0][:, half * 4 : (half + 1) * 4, tch * IC : (tch + 1) * IC],
                        xt_re[:, half * 4 : (half + 1) * 4, tch * IC : (tch + 1) * IC],
                    )

            wq_sb = constp.tile([128, n_ec, F], bf16, tag="wq")
            nc.sync.dma_start(wq_sb[:], wqT.rearrange("(a p) f -> p a f", p=128))
            wv_sb = constp.tile([128, n_ec, F], bf16, tag="wv")
            nc.sync.dma_start(wv_sb[:], wvT.rearrange("(a p) f -> p a f", p=128))
            wo_sb = constp.tile([128, E], bf16, tag="wo")
            nc.sync.dma_start(wo_sb[:], woT)
            bq_sb = constp.tile([128, 1], f32, tag="bq")
            nc.sync.dma_start(bq_sb[:], bq_ap[:])
            bv_sb = constp.tile([128, 1], f32, tag="bv")
            nc.sync.dma_start(bv_sb[:], bv_ap[:])
            tril_sb = constp.tile([128, JC], bf16, tag="tril")
            nc.sync.dma_start(tril_sb[:], tril_ap)
            ones_f32 = constp.tile([128, 1], f32, tag="ones_f32")
            nc.vector.memset(ones_f32[:], 1.0)
            ones_row = ones_f32[:, 0:1].broadcast_to([128, IC])

            for tch in range(n_icb):
                nc.sync.dma_start(
                    xbs[1][:, :, tch * IC : (tch + 1) * IC],
                    xt_re[:, :, T + tch * IC : T + (tch + 1) * IC],
                )

            # ---- persistent activations (all bf16) ----
            qt_sb = persist.tile([128, TT], bf16, tag="qt")    # [f, t]
            kt_sb = persist.tile([128, TT], bf16, tag="kt")    # [f, t]
            # V per token-block: head h occupies cols h*128 .. h*128+64 (V)
            # and h*128+64 .. (h+1)*128 (ones -> replicated denominator)
            v1_sb = persist.tile([128, NTB, 256], bf16, tag="v1")
            yt_sb = persist.tile([128, TT], bf16, tag="yt")    # [e', t] normalized

            for h in range(2):
                nc.gpsimd.memset(v1_sb[:, :, h * 128 + 64 : (h + 1) * 128], 1.0)

            def emit_d(t0):
                # output projection for the 512-token chunk at t0
                for eo in range(E // 128):
                    pso = ps_a.tile([128, IC], f32, tag="psb", name="pso")
                    nc.tensor.matmul(
                        pso[:],
                        wo_sb[:, eo * 128 : (eo + 1) * 128],
                        yt_sb[:, t0 : t0 + IC],
                        start=True,
                        stop=True,
                    )
                    ob = outsp.tile([128, IC], bf16, tag="ob")
                    if eo % 2 == 0:
                        nc.vector.tensor_copy(ob[:], pso[:])
                    else:
                        nc.scalar.activation(
                            ob[:], pso[:], mybir.ActivationFunctionType.Copy
                        )
                    nc.sync.dma_start(
                        out_ap[eo * 128 : (eo + 1) * 128, t0 : t0 + IC], ob[:]
                    )

            d_queue = []
            for b in range(B):
                tb = b * T

                # ---- phase B: project this batch's tokens to QT/KT/VT ----
                xb = xbs[b]
                vt = vtp.tile([128, T], bf16, tag="vt")
                for w_sb, b_sb, kind in (
                    (wk_sb, bk_sb, "k"),
                    (wq_sb, bq_sb, "q"),
                    (wv_sb, bv_sb, "v"),
                ):
                    pss = [
                        ps_a.tile([128, IC], f32, tag="psb", name="psb")
                        for i in range(n_icb)
                    ]
                    for tch in range(n_icb):
                        for ec in range(n_ec):
                            nc.tensor.matmul(
                                pss[tch][:],
                                w_sb[:, ec, :],
                                xb[:, ec, tch * IC : (tch + 1) * IC],
                                start=(ec == 0),
                                stop=(ec == n_ec - 1),
                            )
                        if kind == "q":
                            dst = qt_sb[:, tb + tch * IC : tb + (tch + 1) * IC]
                        elif kind == "k":
                            dst = kt_sb[:, tb + tch * IC : tb + (tch + 1) * IC]
                        else:
                            dst = vt[:, tch * IC : (tch + 1) * IC]
                        nc.vector.scalar_tensor_tensor(
                            dst, pss[tch][:], b_sb[:], ones_row,
                            op0=mybir.AluOpType.add, op1=mybir.AluOpType.mult,
                        )
                        if kind == "v":
                            # V back to natural [t, f] layout with the ones
                            # blocks: one whole-chunk DMA crossbar transpose
                            # (extra out dims fold into the partition dim),
                            # then one strided copy into the per-head slots
                            tci0 = b * (T // 128) + tch * (IC // 128)
                            vtT = work.tile([128, IC // 128, 128], bf16, tag="vtT")
                            nc.sync.dma_start_transpose(
                                vtT[:], vt[:, tch * IC : (tch + 1) * IC]
                            )
                            dst = v1_sb[
                                :, tci0 : tci0 + IC // 128, :
                            ].rearrange("p t (h c) -> p t h c", h=2)
                            nc.vector.tensor_copy(
                                dst[:, :, :, 0:64],
                                vtT[:].rearrange("p t (h c) -> p t h c", h=2),
                            )
                    if kind == "k" and d_queue:
                        # previous batch's last out-projection chunk, deferred
                        # so its PSUM drains overlap this projection round
                        emit_d(d_queue.pop(0))

                # ---- phase C/D interleaved: attention, then out-proj per
                # pair of finished query chunks ----
                for h in range(2):
                    r0 = h * DK
                    yps = [
                        ps_y.tile([128, IC], f32, tag="yp", name=f"yp{i}")
                        for i in range(n_icb)
                    ]

                    def flush_pvs(pend):
                        jc, tci, pts = pend
                        for ic, pt, w, cs in pts:
                            nc.tensor.matmul(
                                yps[ic][:, cs:IC],
                                v1_sb[:, tci, h * 128 : (h + 1) * 128],
                                pt[:, :w],
                                start=(jc == 0),
                                stop=(jc == 4 * ic + 3),
                                skip_group_check=True,
                            )
                        if jc % 4 == 3:
                            # yps[jc//4] is complete: normalize into yt
                            icd = jc // 4
                            den = work.tile([DK, IC], f32, tag="den")
                            nc.vector.tensor_copy(den[:], yps[icd][DK:128, :])
                            rcp = work.tile([DK, IC], f32, tag="rcp")
                            nc.vector.reciprocal_approx_fast(rcp[:], den[:])
                            nc.vector.tensor_mul(
                                yt_sb[r0 : r0 + DK, tb + icd * IC : tb + (icd + 1) * IC],
                                yps[icd][0:DK, :],
                                rcp[:],
                            )
                            if h == 1:
                                d_queue.append(tb + icd * IC)

                    pending = None
                    for jc in range(n_jcb):
                        ic0 = jc // 4
                        o = jc % 4
                        tci = b * (T // 128) + jc
                        pts = []
                        for ic in range(ic0, n_icb):
                            diag = ic == ic0
                            cs = o * JC if diag else 0
                            w = IC - cs
                            q0 = tb + ic * IC + cs
                            st = ps_a.tile([128, IC], f32, tag="psb", name="st")
                            nc.tensor.matmul(
                                st[:, :w],
                                kt_sb[r0 : r0 + DK, tb + jc * JC : tb + (jc + 1) * JC],
                                qt_sb[r0 : r0 + DK, q0 : q0 + w],
                                start=True,
                                stop=True,
                            )
                            pt = ptp.tile([128, IC], bf16, tag="pt")
                            nc.scalar.activation(
                                pt[:, :w], st[:, :w], Act.Exp, scale=0.125
                            )
                            if diag:
                                # mask the diagonal 128x128 sub-block
                                nc.gpsimd.tensor_mul(
                                    pt[:, 0:JC], pt[:, 0:JC], tril_sb[:]
                                )
                            pts.append((ic, pt, w, cs))
                        if d_queue:
                            # drain a deferred out-projection chunk behind a
                            # fresh buffer of QK work
                            emit_d(d_queue.pop(0))
                        if pending is not None:
                            flush_pvs(pending)
                        pending = (jc, tci, pts)
                    flush_pvs(pending)

            while d_queue:
                emit_d(d_queue.pop(0))

            if debug_taps:
                nc.sync.dma_start(dbg_qt[:], qt_sb[:])
                nc.sync.dma_start(dbg_kt[:], kt_sb[:])
                nc.sync.dma_start(dbg_v1[:], v1_sb[:])
                nc.sync.dma_start(dbg_yt[:], yt_sb[:])

    nc.compile()
    return nc


def _get_program():
    if "nc" not in _CACHE:
        _CACHE["nc"] = _build_program()
    return _CACHE["nc"]


def _prepare_in_maps(inputs):
    import ml_dtypes

    bf16 = ml_dtypes.bfloat16
    x = np.ascontiguousarray(
        np.asarray(inputs["x"], dtype=np.float32).reshape(TT, E).T
    ).astype(bf16)
    Wq = np.asarray(inputs["Wq"], dtype=np.float32)
    Wk = np.asarray(inputs["Wk"], dtype=np.float32)
    Wv = np.asarray(inputs["Wv"], dtype=np.float32)
    Wo = np.asarray(inputs["Wo"], dtype=np.float32)
    bq = np.asarray(inputs["bq"], dtype=np.float32)
    bk = np.asarray(inputs["bk"], dtype=np.float32)
    bv = np.asarray(inputs["bv"], dtype=np.float32)

    j = np.arange(JC)[:, None]
    i = np.arange(JC)[None, :]
    tril = (j <= i).astype(bf16)  # key j visible to query i

    in_maps = []
    for c in range(N_CORES):
        sl = slice(c * F, (c + 1) * F)
        in_maps.append(
            {
                "x": x,
                "wqT": np.ascontiguousarray(Wq[sl].T).astype(bf16),
                "wkT": np.ascontiguousarray(Wk[sl].T).astype(bf16),
                "wvT": np.ascontiguousarray(Wv[sl].T).astype(bf16),
                "woT": np.ascontiguousarray(Wo[:, sl].T).astype(bf16),
                "bq": np.ascontiguousarray(bq[sl]),
                "bk": np.ascontiguousarray(bk[sl]),
                "bv": np.ascontiguousarray(bv[sl]),
                "tril": tril,
            }
        )
    return in_maps


def kernel(x, Wq, bq, Wk, bk, Wv, bv, Wo, bo):
    from concourse.bass_utils import run_bass_kernel_spmd

    nc = _get_program()
    bo = np.asarray(bo, dtype=np.float32)
    in_maps = _prepare_in_maps(
        {"x": x, "Wq": Wq, "bq": bq, "Wk": Wk, "bk": bk,
         "Wv": Wv, "bv": bv, "Wo": Wo, "bo": bo}
    )

    res = run_bass_kernel_spmd(nc, in_maps, core_ids=list(range(N_CORES)))
    out = np.zeros((E, TT), dtype=np.float64)
    for c in range(N_CORES):
        out += res.results[c]["partial"].astype(np.float32)
    out = out.T + bo[None, :]
    return out.reshape(B, T, E).astype(np.float32)


# revision 44
# speedup vs baseline: 4.8302x; 2.7121x over previous
"""Causal self-attention (B=2, T=2048, E=1024, H=16, d_k=64) on 8 TRN2 cores.

Tensor-parallel over heads: core c owns heads 2c, 2c+1 (feature slice
c*128:(c+1)*128 of the QKV projections and of the Wo contraction dim).
Each core computes a partial output [1024, 4096] (embd-major, bf16); the
host sums the 8 partials in fp32, adds bo, and transposes back.

All matmul operands are bf16 (1 col/cycle on the PE, half the LDWEIGHTS
cost of fp32r); accumulation stays fp32 in PSUM. Attention blocks above
the causal diagonal are skipped at 128-column granularity, the diagonal
128x128 sub-block is masked with a multiplicative tril mask on GpSimd
after the exp. Softmax denominators are accumulated via a 64-wide ones
block in the PV stationary operand, so the reciprocal lands already
broadcast across partitions 64:128 of the PSUM accumulator
(reciprocal_approx_fast, ~18 bits).

Schedule: per batch, QKV projection rounds (PSUM-bank-resident over the
contraction), V transposed back to token-major via one whole-chunk DMA
crossbar transpose per 512 tokens; attention runs jc-pipelined (PV of
key-block jc-1 issues behind QK/exp of block jc, so the PE never waits
on the scalar engine), and each 512-token output-projection chunk is
deferred one key-block behind its normalize and drained inside the
attention stream. The tensor engine runs ~90% busy inside the kernel
span; the remaining gap to the 115us stream-cycle floor is the chip's
k-of-8 PE duty-cycle throttle (50% steady state after a ~25us boost
window, re-armed by idle periods).
"""

import numpy as np

B = 2
T = 2048
E = 1024
F = 128          # per-core QKV features (2 heads x 64)
DK = 64
N_CORES = 8
TT = B * T       # flattened tokens
IC = 512         # query chunk (PSUM bank width in fp32)
JC = 128         # key block
NTB = TT // JC   # 32 token blocks of 128

_CACHE = {}


def _build_program(debug_taps=False):
    import concourse.mybir as mybir
    import concourse.tile as tile
    from concourse import bacc

    f32 = mybir.dt.float32
    bf16 = mybir.dt.bfloat16
    Act = mybir.ActivationFunctionType

    n_ec = E // 128          # 8 contraction chunks for the projections
    n_icb = T // IC          # 4 query chunks per batch
    n_jcb = T // JC          # 16 key blocks per batch

    nc = bacc.Bacc("TRN2", target_bir_lowering=False, debug=False)

    # x arrives already transposed on the host: [embd, t_total]
    x_ap = nc.dram_tensor("x", [E, TT], bf16, kind="ExternalInput").ap()
    wqT = nc.dram_tensor("wqT", [E, F], bf16, kind="ExternalInput").ap()
    wkT = nc.dram_tensor("wkT", [E, F], bf16, kind="ExternalInput").ap()
    wvT = nc.dram_tensor("wvT", [E, F], bf16, kind="ExternalInput").ap()
    woT = nc.dram_tensor("woT", [F, E], bf16, kind="ExternalInput").ap()
    bq_ap = nc.dram_tensor("bq", [F], f32, kind="ExternalInput").ap()
    bk_ap = nc.dram_tensor("bk", [F], f32, kind="ExternalInput").ap()
    bv_ap = nc.dram_tensor("bv", [F], f32, kind="ExternalInput").ap()
    tril_ap = nc.dram_tensor("tril", [JC, JC], bf16, kind="ExternalInput").ap()
    out_ap = nc.dram_tensor("partial", [E, TT], bf16, kind="ExternalOutput").ap()
    if debug_taps:
        dbg_qt = nc.dram_tensor("dbg_qt", [128, TT], bf16, kind="ExternalOutput").ap()
        dbg_kt = nc.dram_tensor("dbg_kt", [128, TT], bf16, kind="ExternalOutput").ap()
        dbg_v1 = nc.dram_tensor("dbg_v1", [128, NTB, 256], bf16, kind="ExternalOutput").ap()
        dbg_yt = nc.dram_tensor("dbg_yt", [128, TT], bf16, kind="ExternalOutput").ap()

    with tile.TileContext(nc) as tc:
        with (
            tc.tile_pool(name="const", bufs=1) as constp,
            tc.tile_pool(name="persist", bufs=1) as persist,
            tc.tile_pool(name="xb", bufs=2) as xbp,
            tc.tile_pool(name="vt", bufs=2) as vtp,
            tc.tile_pool(name="pt", bufs=12) as ptp,
            tc.tile_pool(name="work", bufs=4) as work,
            tc.tile_pool(name="outs", bufs=8) as outsp,
            tc.tile_pool(name="ps_a", bufs=4, space="PSUM") as ps_a,
            tc.tile_pool(name="ps_y", bufs=4, space="PSUM") as ps_y,
        ):
            # ---- constants; k-projection weights first, then batch-0 x,
            # so the first projection round unblocks as early as possible ----
            wk_sb = constp.tile([128, n_ec, F], bf16, tag="wk")
            nc.sync.dma_start(wk_sb[:], wkT.rearrange("(a p) f -> p a f", p=128))
            bk_sb = constp.tile([128, 1], f32, tag="bk")
            nc.sync.dma_start(bk_sb[:], bk_ap[:])

            xt_re = x_ap.rearrange("(a p) t -> p a t", p=128)
            xbs = []
            for b in range(B):
                xb = xbp.tile([128, n_ec, T], bf16, tag="xb", name=f"xb{b}")
                xbs.append(xb)
            # batch-0 first chunk split along ec so the first matmul
            # unblocks early
            for piece in range(4):
                nc.sync.dma_start(
                    xbs[0][:, piece * 2 : (piece + 1) * 2, 0:IC],
                    xt_re[:, piece * 2 : (piece + 1) * 2, 0:IC],
                )
            for tch in range(1, n_icb):
                for half in range(2):
                    nc.sync.dma_start(
                        xbs[0][:, half * 4 : (half + 1) * 4, tch * IC : (tch + 1) * IC],
                        xt_re[:, half * 4 : (half + 1) * 4, tch * IC : (tch + 1) * IC],
                    )

            wq_sb = constp.tile([128, n_ec, F], bf16, tag="wq")
            nc.sync.dma_start(wq_sb[:], wqT.rearrange("(a p) f -> p a f", p=128))
            wv_sb = constp.tile([128, n_ec, F], bf16, tag="wv")
            nc.sync.dma_start(wv_sb[:], wvT.rearrange("(a p) f -> p a f", p=128))
            wo_sb = constp.tile([128, E], bf16, tag="wo")
            nc.sync.dma_start(wo_sb[:], woT)
            bq_sb = constp.tile([128, 1], f32, tag="bq")
            nc.sync.dma_start(bq_sb[:], bq_ap[:])
            bv_sb = constp.tile([128, 1], f32, tag="bv")
            nc.sync.dma_start(bv_sb[:], bv_ap[:])
            tril_sb = constp.tile([128, JC], bf16, tag="tril")
            nc.sync.dma_start(tril_sb[:], tril_ap)
            ones_f32 = constp.tile([128, 1], f32, tag="ones_f32")
            nc.vector.memset(ones_f32[:], 1.0)
            ones_row = ones_f32[:, 0:1].broadcast_to([128, IC])

            for tch in range(n_icb):
                nc.sync.dma_start(
                    xbs[1][:, :, tch * IC : (tch + 1) * IC],
                    xt_re[:, :, T + tch * IC : T + (tch + 1) * IC],
                )

            # ---- persistent activations (all bf16) ----
            qt_sb = persist.tile([128, TT], bf16, tag="qt")    # [f, t]
            kt_sb = persist.tile([128, TT], bf16, tag="kt")    # [f, t]
            # V per token-block: head h occupies cols h*128 .. h*128+64 (V)
            # and h*128+64 .. (h+1)*128 (ones -> replicated denominator)
            v1_sb = persist.tile([128, NTB, 256], bf16, tag="v1")
            yt_sb = persist.tile([128, TT], bf16, tag="yt")    # [e', t] normalized

            for h in range(2):
                nc.gpsimd.memset(v1_sb[:, :, h * 128 + 64 : (h + 1) * 128], 1.0)

            def emit_d(t0):
                # output projection for the 512-token chunk at t0
                for eo in range(E // 128):
                    pso = ps_a.tile([128, IC], f32, tag="psb", name="pso")
                    nc.tensor.matmul(
                        pso[:],
                        wo_sb[:, eo * 128 : (eo + 1) * 128],
                        yt_sb[:, t0 : t0 + IC],
                        start=True,
                        stop=True,
                    )
                    ob = outsp.tile([128, IC], bf16, tag="ob")
                    if eo % 2 == 0:
                        nc.vector.tensor_copy(ob[:], pso[:])
                    else:
                        nc.scalar.activation(
                            ob[:], pso[:], mybir.ActivationFunctionType.Copy
                        )
                    nc.sync.dma_start(
                        out_ap[eo * 128 : (eo + 1) * 128, t0 : t0 + IC], ob[:]
                    )

            d_queue = []
            for b in range(B):
                tb = b * T

                # ---- phase B: project this batch's tokens to QT/KT/VT ----
                xb = xbs[b]
                vt = vtp.tile([128, T], bf16, tag="vt")
                for w_sb, b_sb, kind in (
                    (wk_sb, bk_sb, "k"),
                    (wq_sb, bq_sb, "q"),
                    (wv_sb, bv_sb, "v"),
                ):
                    pss = [
                        ps_a.tile([128, IC], f32, tag="psb", name="psb")
                        for i in range(n_icb)
                    ]
                    for tch in range(n_icb):
                        for ec in range(n_ec):
                            nc.tensor.matmul(
                                pss[tch][:],
                                w_sb[:, ec, :],
                                xb[:, ec, tch * IC : (tch + 1) * IC],
                                start=(ec == 0),
                                stop=(ec == n_ec - 1),
                            )
                        if kind == "q":
                            dst = qt_sb[:, tb + tch * IC : tb + (tch + 1) * IC]
                        elif kind == "k":
                            dst = kt_sb[:, tb + tch * IC : tb + (tch + 1) * IC]
                        else:
                            dst = vt[:, tch * IC : (tch + 1) * IC]
                        nc.vector.scalar_tensor_tensor(
                            dst, pss[tch][:], b_sb[:], ones_row,
                            op0=mybir.AluOpType.add, op1=mybir.AluOpType.mult,
                        )
                        if kind == "v":
                            # V back to natural [t, f] layout with the ones
                            # blocks: one whole-chunk DMA crossbar transpose
                            # (extra out dims fold into the partition dim),
                            # then one strided copy into the per-head slots
                            tci0 = b * (T // 128) + tch * (IC // 128)
                            vtT = work.tile([128, IC // 128, 128], bf16, tag="vtT")
                            nc.sync.dma_start_transpose(
                                vtT[:], vt[:, tch * IC : (tch + 1) * IC]
                            )
                            dst = v1_sb[
                                :, tci0 : tci0 + IC // 128, :
                            ].rearrange("p t (h c) -> p t h c", h=2)
                            nc.vector.tensor_copy(
                                dst[:, :, :, 0:64],
                                vtT[:].rearrange("p t (h c) -> p t h c", h=2),
                            )
                    if kind == "k" and d_queue:
                        # previous batch's last out-projection chunk, deferred
                        # so its PSUM drains overlap this projection round
                        emit_d(d_queue.pop(0))

                # ---- phase C/D interleaved: attention, then out-proj per
                # pair of finished query chunks ----
                for h in range(2):
                    r0 = h * DK
                    yps = [
                        ps_y.tile([128, IC], f32, tag="yp", name=f"yp{i}")
                        for i in range(n_icb)
                    ]

                    def flush_pvs(pend):
                        jc, tci, pts = pend
                        for ic, pt, w, cs in pts:
                            nc.tensor.matmul(
                                yps[ic][:, cs:IC],
                                v1_sb[:, tci, h * 128 : (h + 1) * 128],
                                pt[:, :w],
                                start=(jc == 0),
                                stop=(jc == 4 * ic + 3),
                                skip_group_check=True,
                            )
                        if jc % 4 == 3:
                            # yps[jc//4] is complete: normalize into yt
                            icd = jc // 4
                            den = work.tile([DK, IC], f32, tag="den")
                            nc.vector.tensor_copy(den[:], yps[icd][DK:128, :])
                            rcp = work.tile([DK, IC], f32, tag="rcp")
                            nc.vector.reciprocal_approx_fast(rcp[:], den[:])
                            nc.vector.tensor_mul(
                                yt_sb[r0 : r0 + DK, tb + icd * IC : tb + (icd + 1) * IC],
                                yps[icd][0:DK, :],
                                rcp[:],
                            )
                            if h == 1:
                                d_queue.append(tb + icd * IC)

                    pending = None
                    for jc in range(n_jcb):
                        ic0 = jc // 4
                        o = jc % 4
                        tci = b * (T // 128) + jc
                        pts = []
                        for ic in range(ic0, n_icb):
                            diag = ic == ic0
                            cs = o * JC if diag else 0
                            w = IC - cs
                            q0 = tb + ic * IC + cs
                            st = ps_a.tile([128, IC], f32, tag="psb", name="st")
                            nc.tensor.matmul(
                                st[:, :w],
                                kt_sb[r0 : r0 + DK, tb + jc * JC : tb + (jc + 1) * JC],
                                qt_sb[r0 : r0 + DK, q0 : q0 + w],
                                start=True,
                                stop=True,
                            )
                            pt = ptp.tile([128, IC], bf16, tag="pt")
                            nc.scalar.activation(
                                pt[:, :w], st[:, :w], Act.Exp, scale=0.125
                            )
                            if diag:
                                # mask the diagonal 128x128 sub-block
                                nc.gpsimd.tensor_mul(
                                    pt[:, 0:JC], pt[:, 0:JC], tril_sb[:]
                                )
                            pts.append((ic, pt, w, cs))
                        if d_queue:
                            # drain a deferred out-projection chunk behind a
                            # fresh buffer of QK work
                            emit_d(d_queue.pop(0))
                        if pending is not None:
                            flush_pvs(pending)
                        pending = (jc, tci, pts)
                    flush_pvs(pending)

            while d_queue:
                emit_d(d_queue.pop(0))

            if debug_taps:
                nc.sync.dma_start(dbg_qt[:], qt_sb[:])
                nc.sync.dma_start(dbg_kt[:], kt_sb[:])
                nc.sync.dma_start(dbg_v1[:], v1_sb[:])
                nc.sync.dma_start(dbg_yt[:], yt_sb[:])

    nc.compile()
    return nc


def _get_program():
    if "nc" not in _CACHE:
        _CACHE["nc"] = _build_program()
    return _CACHE["nc"]


def _prepare_in_maps(inputs):
    import ml_dtypes

    bf16 = ml_dtypes.bfloat16
    x = np.ascontiguousarray(
        np.asarray(inputs["x"], dtype=np.float32).reshape(TT, E).T
    ).astype(bf16)
    Wq = np.asarray(inputs["Wq"], dtype=np.float32)
    Wk = np.asarray(inputs["Wk"], dtype=np.float32)
    Wv = np.asarray(inputs["Wv"], dtype=np.float32)
    Wo = np.asarray(inputs["Wo"], dtype=np.float32)
    bq = np.asarray(inputs["bq"], dtype=np.float32)
    bk = np.asarray(inputs["bk"], dtype=np.float32)
    bv = np.asarray(inputs["bv"], dtype=np.float32)

    j = np.arange(JC)[:, None]
    i = np.arange(JC)[None, :]
    tril = (j <= i).astype(bf16)  # key j visible to query i

    in_maps = []
    for c in range(N_CORES):
        sl = slice(c * F, (c + 1) * F)
        in_maps.append(
            {
                "x": x,
                "wqT": np.ascontiguousarray(Wq[sl].T).astype(bf16),
                "wkT": np.ascontiguousarray(Wk[sl].T).astype(bf16),
                "wvT": np.ascontiguousarray(Wv[sl].T).astype(bf16),
                "woT": np.ascontiguousarray(Wo[:, sl].T).astype(bf16),
                "bq": np.ascontiguousarray(bq[sl]),
                "bk": np.ascontiguousarray(bk[sl]),
                "bv": np.ascontiguousarray(bv[sl]),
                "tril": tril,
            }
        )
    return in_maps


def kernel(x, Wq, bq, Wk, bk, Wv, bv, Wo, bo):
    from concourse.bass_utils import run_bass_kernel_spmd

    nc = _get_program()
    bo = np.asarray(bo, dtype=np.float32)
    in_maps = _prepare_in_maps(
        {"x": x, "Wq": Wq, "bq": bq, "Wk": Wk, "bk": bk,
         "Wv": Wv, "bv": bv, "Wo": Wo, "bo": bo}
    )

    res = run_bass_kernel_spmd(nc, in_maps, core_ids=list(range(N_CORES)))
    out = np.zeros((E, TT), dtype=np.float64)
    for c in range(N_CORES):
        out += res.results[c]["partial"].astype(np.float32)
    out = out.T + bo[None, :]
    return out.reshape(B, T, E).astype(np.float32)


# revision 49
# speedup vs baseline: 4.8571x; 1.0056x over previous
"""Causal self-attention (B=2, T=2048, E=1024, H=16, d_k=64) on 8 TRN2 cores.

Tensor-parallel over heads: core c owns heads 2c, 2c+1 (feature slice
c*128:(c+1)*128 of the QKV projections and of the Wo contraction dim).
Each core computes a partial output [1024, 4096] (embd-major, bf16); the
host sums the 8 partials in fp32, adds bo, and transposes back.

All matmul operands are bf16 (1 col/cycle on the PE, half the LDWEIGHTS
cost of fp32r); accumulation stays fp32 in PSUM. Attention blocks above
the causal diagonal are skipped at 128-column granularity, the diagonal
128x128 sub-block is masked with a multiplicative tril mask on GpSimd
after the exp. Softmax denominators are accumulated via a 64-wide ones
block in the PV stationary operand, so the reciprocal lands already
broadcast across partitions 64:128 of the PSUM accumulator
(reciprocal_approx_fast, ~18 bits).

Schedule: per batch, QKV projection rounds (PSUM-bank-resident over the
contraction), V transposed back to token-major via one whole-chunk DMA
crossbar transpose per 512 tokens; attention runs jc-pipelined (PV of
key-block jc-1 issues behind QK/exp of block jc, so the PE never waits
on the scalar engine), and each 512-token output-projection chunk is
deferred one key-block behind its normalize and drained inside the
attention stream. The tensor engine runs ~90% busy inside the kernel
span; the remaining gap to the 115us stream-cycle floor is the chip's
k-of-8 PE duty-cycle throttle (50% steady state after a ~25us boost
window, re-armed by idle periods).
"""

import numpy as np

B = 2
T = 2048
E = 1024
F = 128          # per-core QKV features (2 heads x 64)
DK = 64
N_CORES = 8
TT = B * T       # flattened tokens
IC = 512         # query chunk (PSUM bank width in fp32)
JC = 128         # key block
NTB = TT // JC   # 32 token blocks of 128

_CACHE = {}


def _build_program(debug_taps=False):
    import concourse.mybir as mybir
    import concourse.tile as tile
    from concourse import bacc

    f32 = mybir.dt.float32
    bf16 = mybir.dt.bfloat16
    Act = mybir.ActivationFunctionType

    n_ec = E // 128          # 8 contraction chunks for the projections
    n_icb = T // IC          # 4 query chunks per batch
    n_jcb = T // JC          # 16 key blocks per batch

    nc = bacc.Bacc("TRN2", target_bir_lowering=False, debug=False)

    # x arrives already transposed on the host: [embd, t_total]
    x_ap = nc.dram_tensor("x", [E, TT], bf16, kind="ExternalInput").ap()
    wqT = nc.dram_tensor("wqT", [E, F], bf16, kind="ExternalInput").ap()
    wkT = nc.dram_tensor("wkT", [E, F], bf16, kind="ExternalInput").ap()
    wvT = nc.dram_tensor("wvT", [E, F], bf16, kind="ExternalInput").ap()
    woT = nc.dram_tensor("woT", [F, E], bf16, kind="ExternalInput").ap()
    bq_ap = nc.dram_tensor("bq", [F], f32, kind="ExternalInput").ap()
    bk_ap = nc.dram_tensor("bk", [F], f32, kind="ExternalInput").ap()
    bv_ap = nc.dram_tensor("bv", [F], f32, kind="ExternalInput").ap()
    tril_ap = nc.dram_tensor("tril", [JC, JC], bf16, kind="ExternalInput").ap()
    out_ap = nc.dram_tensor("partial", [E, TT], bf16, kind="ExternalOutput").ap()
    if debug_taps:
        dbg_qt = nc.dram_tensor("dbg_qt", [128, TT], bf16, kind="ExternalOutput").ap()
        dbg_kt = nc.dram_tensor("dbg_kt", [128, TT], bf16, kind="ExternalOutput").ap()
        dbg_v1 = nc.dram_tensor("dbg_v1", [128, NTB, 256], bf16, kind="ExternalOutput").ap()
        dbg_yt = nc.dram_tensor("dbg_yt", [128, TT], bf16, kind="ExternalOutput").ap()

    with tile.TileContext(nc) as tc:
        with (
            tc.tile_pool(name="const", bufs=1) as constp,
            tc.tile_pool(name="persist", bufs=1) as persist,
            tc.tile_pool(name="xb", bufs=2) as xbp,
            tc.tile_pool(name="vt", bufs=2) as vtp,
            tc.tile_pool(name="pt", bufs=12) as ptp,
            tc.tile_pool(name="work", bufs=4) as work,
            tc.tile_pool(name="outs", bufs=8) as outsp,
            tc.tile_pool(name="ps_a", bufs=4, space="PSUM") as ps_a,
            tc.tile_pool(name="ps_y", bufs=4, space="PSUM") as ps_y,
        ):
            # ---- constants; k-projection weights first, then batch-0 x,
            # so the first projection round unblocks as early as possible ----
            wk_sb = constp.tile([128, n_ec, F], bf16, tag="wk")
            nc.sync.dma_start(wk_sb[:], wkT.rearrange("(a p) f -> p a f", p=128))
            bk_sb = constp.tile([128, 1], f32, tag="bk")
            nc.sync.dma_start(bk_sb[:], bk_ap[:])

            xt_re = x_ap.rearrange("(a p) t -> p a t", p=128)
            xbs = []
            for b in range(B):
                xb = xbp.tile([128, n_ec, T], bf16, tag="xb", name=f"xb{b}")
                xbs.append(xb)
            # batch-0 first chunk split along ec so the first matmul
            # unblocks early
            for piece in range(4):
                nc.sync.dma_start(
                    xbs[0][:, piece * 2 : (piece + 1) * 2, 0:IC],
                    xt_re[:, piece * 2 : (piece + 1) * 2, 0:IC],
                )
            for tch in range(1, n_icb):
                for half in range(2):
                    nc.sync.dma_start(
                        xbs[0][:, half * 4 : (half + 1) * 4, tch * IC : (tch + 1) * IC],
                        xt_re[:, half * 4 : (half + 1) * 4, tch * IC : (tch + 1) * IC],
                    )

            wq_sb = constp.tile([128, n_ec, F], bf16, tag="wq")
            nc.sync.dma_start(wq_sb[:], wqT.rearrange("(a p) f -> p a f", p=128))
            wv_sb = constp.tile([128, n_ec, F], bf16, tag="wv")
            nc.sync.dma_start(wv_sb[:], wvT.rearrange("(a p) f -> p a f", p=128))
            wo_sb = constp.tile([128, E], bf16, tag="wo")
            nc.sync.dma_start(wo_sb[:], woT)
            bq_sb = constp.tile([128, 1], f32, tag="bq")
            nc.sync.dma_start(bq_sb[:], bq_ap[:])
            bv_sb = constp.tile([128, 1], f32, tag="bv")
            nc.sync.dma_start(bv_sb[:], bv_ap[:])
            tril_sb = constp.tile([128, JC], bf16, tag="tril")
            nc.sync.dma_start(tril_sb[:], tril_ap)
            ones_f32 = constp.tile([128, 1], f32, tag="ones_f32")
            nc.vector.memset(ones_f32[:], 1.0)
            ones_row = ones_f32[:, 0:1].broadcast_to([128, IC])

            for tch in range(n_icb):
                nc.sync.dma_start(
                    xbs[1][:, :, tch * IC : (tch + 1) * IC],
                    xt_re[:, :, T + tch * IC : T + (tch + 1) * IC],
                )

            # ---- persistent activations (all bf16) ----
            qt_sb = persist.tile([128, TT], bf16, tag="qt")    # [f, t]
            kt_sb = persist.tile([128, TT], bf16, tag="kt")    # [f, t]
            # V per token-block: head h occupies cols h*128 .. h*128+64 (V)
            # and h*128+64 .. (h+1)*128 (ones -> replicated denominator)
            v1_sb = persist.tile([128, NTB, 256], bf16, tag="v1")
            yt_sb = persist.tile([128, TT], bf16, tag="yt")    # [e', t] normalized

            for h in range(2):
                nc.gpsimd.memset(v1_sb[:, :, h * 128 + 64 : (h + 1) * 128], 1.0)

            def emit_d(t0, last=False):
                # output projection for the 512-token chunk at t0
                for eo in range(E // 128):
                    pso = ps_a.tile([128, IC], f32, tag="psb", name="pso")
                    nc.tensor.matmul(
                        pso[:],
                        wo_sb[:, eo * 128 : (eo + 1) * 128],
                        yt_sb[:, t0 : t0 + IC],
                        start=True,
                        stop=True,
                    )
                    ob = outsp.tile([128, IC], bf16, tag="ob")
                    if eo % 2 == 0:
                        nc.vector.tensor_copy(ob[:], pso[:])
                    else:
                        nc.scalar.activation(
                            ob[:], pso[:], mybir.ActivationFunctionType.Copy
                        )
                    # for the final chunk the scalar engine is idle: split
                    # the out-DMA issues across both HWDGE queues
                    dma_eng = nc.scalar if (last and eo % 2) else nc.sync
                    dma_eng.dma_start(
                        out_ap[eo * 128 : (eo + 1) * 128, t0 : t0 + IC], ob[:]
                    )

            d_queue = []
            for b in range(B):
                tb = b * T

                # ---- phase B: project this batch's tokens to QT/KT/VT ----
                xb = xbs[b]
                vt = vtp.tile([128, T], bf16, tag="vt")
                v_copies = []
                for w_sb, b_sb, kind in (
                    (wk_sb, bk_sb, "k"),
                    (wq_sb, bq_sb, "q"),
                    (wv_sb, bv_sb, "v"),
                ):
                    pss = [
                        ps_a.tile([128, IC], f32, tag="psb", name="psb")
                        for i in range(n_icb)
                    ]
                    for tch in range(n_icb):
                        for ec in range(n_ec):
                            nc.tensor.matmul(
                                pss[tch][:],
                                w_sb[:, ec, :],
                                xb[:, ec, tch * IC : (tch + 1) * IC],
                                start=(ec == 0),
                                stop=(ec == n_ec - 1),
                            )
                        if kind == "q":
                            dst = qt_sb[:, tb + tch * IC : tb + (tch + 1) * IC]
                        elif kind == "k":
                            dst = kt_sb[:, tb + tch * IC : tb + (tch + 1) * IC]
                        else:
                            dst = vt[:, tch * IC : (tch + 1) * IC]
                        nc.vector.scalar_tensor_tensor(
                            dst, pss[tch][:], b_sb[:], ones_row,
                            op0=mybir.AluOpType.add, op1=mybir.AluOpType.mult,
                        )
                        if kind == "v":
                            # V back to natural [t, f] layout: one whole-chunk
                            # DMA crossbar transpose (extra out dims fold into
                            # the partition dim); the strided copies into the
                            # per-head slots are deferred below so they cannot
                            # head-of-line-block the PSUM drains on DVE
                            tci0 = b * (T // 128) + tch * (IC // 128)
                            vtT = work.tile([128, IC // 128, 128], bf16, tag="vtT")
                            nc.sync.dma_start_transpose(
                                vtT[:], vt[:, tch * IC : (tch + 1) * IC]
                            )
                            v_copies.append((tci0, vtT))
                    if kind == "v":
                        for tci0, vtT in v_copies:
                            dst = v1_sb[
                                :, tci0 : tci0 + IC // 128, :
                            ].rearrange("p t (h c) -> p t h c", h=2)
                            nc.vector.tensor_copy(
                                dst[:, :, :, 0:64],
                                vtT[:].rearrange("p t (h c) -> p t h c", h=2),
                            )
                        v_copies.clear()
                    if kind == "k" and d_queue:
                        # previous batch's last out-projection chunk, deferred
                        # so its PSUM drains overlap this projection round
                        emit_d(d_queue.pop(0))

                # ---- phase C/D interleaved: attention, then out-proj per
                # pair of finished query chunks ----
                for h in range(2):
                    r0 = h * DK
                    yps = [
                        ps_y.tile([128, IC], f32, tag="yp", name=f"yp{i}")
                        for i in range(n_icb)
                    ]

                    def flush_pvs(pend):
                        jc, tci, pts = pend
                        for ic, pt, w, cs in pts:
                            nc.tensor.matmul(
                                yps[ic][:, cs:IC],
                                v1_sb[:, tci, h * 128 : (h + 1) * 128],
                                pt[:, :w],
                                start=(jc == 0),
                                stop=(jc == 4 * ic + 3),
                                skip_group_check=True,
                            )
                        if jc % 4 == 3:
                            # yps[jc//4] is complete: normalize into yt
                            icd = jc // 4
                            den = work.tile([DK, IC], f32, tag="den")
                            nc.vector.tensor_copy(den[:], yps[icd][DK:128, :])
                            rcp = work.tile([DK, IC], f32, tag="rcp")
                            nc.vector.reciprocal_approx_fast(rcp[:], den[:])
                            nc.vector.tensor_mul(
                                yt_sb[r0 : r0 + DK, tb + icd * IC : tb + (icd + 1) * IC],
                                yps[icd][0:DK, :],
                                rcp[:],
                            )
                            if h == 1:
                                d_queue.append(tb + icd * IC)

                    pending = None
                    for jc in range(n_jcb):
                        ic0 = jc // 4
                        o = jc % 4
                        tci = b * (T // 128) + jc
                        pts = []
                        for ic in range(ic0, n_icb):
                            diag = ic == ic0
                            cs = o * JC if diag else 0
                            w = IC - cs
                            q0 = tb + ic * IC + cs
                            st = ps_a.tile([128, IC], f32, tag="psb", name="st")
                            nc.tensor.matmul(
                                st[:, :w],
                                kt_sb[r0 : r0 + DK, tb + jc * JC : tb + (jc + 1) * JC],
                                qt_sb[r0 : r0 + DK, q0 : q0 + w],
                                start=True,
                                stop=True,
                            )
                            pt = ptp.tile([128, IC], bf16, tag="pt")
                            nc.scalar.activation(
                                pt[:, :w], st[:, :w], Act.Exp, scale=0.125
                            )
                            if diag:
                                # mask the diagonal 128x128 sub-block
                                nc.gpsimd.tensor_mul(
                                    pt[:, 0:JC], pt[:, 0:JC], tril_sb[:]
                                )
                            pts.append((ic, pt, w, cs))
                        if d_queue:
                            # drain a deferred out-projection chunk behind a
                            # fresh buffer of QK work
                            emit_d(d_queue.pop(0))
                        if pending is not None:
                            flush_pvs(pending)
                        pending = (jc, tci, pts)
                    flush_pvs(pending)

            while d_queue:
                emit_d(d_queue.pop(0), last=True)

            if debug_taps:
                nc.sync.dma_start(dbg_qt[:], qt_sb[:])
                nc.sync.dma_start(dbg_kt[:], kt_sb[:])
                nc.sync.dma_start(dbg_v1[:], v1_sb[:])
                nc.sync.dma_start(dbg_yt[:], yt_sb[:])

    nc.compile()
    return nc


def _get_program():
    if "nc" not in _CACHE:
        _CACHE["nc"] = _build_program()
    return _CACHE["nc"]


def _prepare_in_maps(inputs):
    import ml_dtypes

    bf16 = ml_dtypes.bfloat16
    x = np.ascontiguousarray(
        np.asarray(inputs["x"], dtype=np.float32).reshape(TT, E).T
    ).astype(bf16)
    Wq = np.asarray(inputs["Wq"], dtype=np.float32)
    Wk = np.asarray(inputs["Wk"], dtype=np.float32)
    Wv = np.asarray(inputs["Wv"], dtype=np.float32)
    Wo = np.asarray(inputs["Wo"], dtype=np.float32)
    bq = np.asarray(inputs["bq"], dtype=np.float32)
    bk = np.asarray(inputs["bk"], dtype=np.float32)
    bv = np.asarray(inputs["bv"], dtype=np.float32)

    j = np.arange(JC)[:, None]
    i = np.arange(JC)[None, :]
    tril = (j <= i).astype(bf16)  # key j visible to query i

    in_maps = []
    for c in range(N_CORES):
        sl = slice(c * F, (c + 1) * F)
        in_maps.append(
            {
                "x": x,
                "wqT": np.ascontiguousarray(Wq[sl].T).astype(bf16),
                "wkT": np.ascontiguousarray(Wk[sl].T).astype(bf16),
                "wvT": np.ascontiguousarray(Wv[sl].T).astype(bf16),
                "woT": np.ascontiguousarray(Wo[:, sl].T).astype(bf16),
                "bq": np.ascontiguousarray(bq[sl]),
                "bk": np.ascontiguousarray(bk[sl]),
                "bv": np.ascontiguousarray(bv[sl]),
                "tril": tril,
            }
        )
    return in_maps


def kernel(x, Wq, bq, Wk, bk, Wv, bv, Wo, bo):
    from concourse.bass_utils import run_bass_kernel_spmd

    nc = _get_program()
    bo = np.asarray(bo, dtype=np.float32)
    in_maps = _prepare_in_maps(
        {"x": x, "Wq": Wq, "bq": bq, "Wk": Wk, "bk": bk,
         "Wv": Wv, "bv": bv, "Wo": Wo, "bo": bo}
    )

    res = run_bass_kernel_spmd(nc, in_maps, core_ids=list(range(N_CORES)))
    out = np.zeros((E, TT), dtype=np.float64)
    for c in range(N_CORES):
        out += res.results[c]["partial"].astype(np.float32)
    out = out.T + bo[None, :]
    return out.reshape(B, T, E).astype(np.float32)
